# revision 14
# baseline (speedup 1.0000x reference)
"""Trainium2 Bass kernel for nn_GaussianActor (moe_routing).

Strategy (v2):
  - Data parallel over batch across 8 cores; weights replicated except
    per-(core,tile) gathered head weights.
  - Host folds W3 into the per-stage heads (no activation between them):
      What[s] = W3 @ Wh[s],  bhat[s] = b3 @ Wh[s] + bh[s]
  - Host folds the LayerNorm mean subtraction into W0 as a rank-1 update:
      W0c = W0 - rowmean(W0)*1^T, b0c = b0 - mean(b0)
    so h = x @ W0c + b0c is already centered; only the variance needs
    computing on device (squares on vector engine, one ones-vector matmul
    pair on PE, reciprocal_approx_fast + sqrt for rstd).
  - Exact packing: samples globally sorted by stage -> 64 tiles of 512
    columns; <=7 tiles span a stage boundary. Every core gets 8 tiles;
    tile 7 is the (at most one) "dual" tile that computes 2 heads; the
    host gathers per-tile head weights, and selects per-column outputs.
  - Device: feature-major activations (features on partitions, batch on
    free axis), fp32r matmuls, Prelu (parametric relu) activations so the
    scalar engine keeps a single act table (sqrt_and_others) resident.
"""

import numpy as np

import concourse.tile as tile
from concourse import bacc, mybir
from concourse import bass_utils
from concourse.alu_op_type import AluOpType

dt = mybir.dt
AF = mybir.ActivationFunctionType

B = 32768
OBS = 512
HID = 1024
A2 = 128          # 2 * action_dim
NSTAGE = 8
NCORES = 8

SEG = 512         # columns per tile
NT = 8            # tiles per core
COLS = NT * SEG   # 4096 columns per core
NHEAD = NT + 1    # 8 primary head slots + 1 secondary (dual tile)

EPS = 1e-5
SLOPE = 0.01
LOG_STD_MIN, LOG_STD_MAX = -20.0, 2.0

KO = OBS // 128   # 4 k-blocks for layer 0
KH = HID // 128   # 8 k-blocks for hidden layers
MH = HID // 128   # 8 m-blocks of hidden features

_CACHE = {}


def _build_nc():
    nc = bacc.Bacc("TRN2", target_bir_lowering=False, debug=False,
                   num_devices=NCORES)

    obsT = nc.dram_tensor("obsT", [OBS, COLS], dt.float32r, kind="ExternalInput").ap()
    w0 = nc.dram_tensor("w0", [OBS, HID], dt.float32r, kind="ExternalInput").ap()
    w1 = nc.dram_tensor("w1", [HID, HID], dt.float32r, kind="ExternalInput").ap()
    w2 = nc.dram_tensor("w2", [HID, HID], dt.float32r, kind="ExternalInput").ap()
    wh = nc.dram_tensor("wh", [HID, NHEAD * A2], dt.float32r, kind="ExternalInput").ap()
    b0d = nc.dram_tensor("b0d", [128, MH], dt.float32, kind="ExternalInput").ap()
    b1d = nc.dram_tensor("b1d", [128, MH], dt.float32, kind="ExternalInput").ap()
    b2d = nc.dram_tensor("b2d", [128, MH], dt.float32, kind="ExternalInput").ap()
    lnwd = nc.dram_tensor("lnwd", [128, MH], dt.float32, kind="ExternalInput").ap()
    lnbd = nc.dram_tensor("lnbd", [128, MH], dt.float32, kind="ExternalInput").ap()
    bhd = nc.dram_tensor("bhd", [128, NHEAD], dt.float32, kind="ExternalInput").ap()
    onesd = nc.dram_tensor("onesd", [128, 1], dt.float32r, kind="ExternalInput").ap()
    epsd = nc.dram_tensor("epsd", [1, 1], dt.float32, kind="ExternalInput").ap()
    onesrd = nc.dram_tensor("onesrd", [1, 128], dt.float32r, kind="ExternalInput").ap()

    out_main = nc.dram_tensor("out_main", [A2, COLS], dt.float32,
                              kind="ExternalOutput").ap()
    out_sec = nc.dram_tensor("out_sec", [A2, SEG], dt.float32,
                             kind="ExternalOutput").ap()

    with tile.TileContext(nc) as tc:
        with tc.tile_pool(name="w", bufs=1) as wp, \
             tc.tile_pool(name="acts", bufs=1) as ap_, \
             tc.tile_pool(name="ps", bufs=6, space="PSUM") as pm, \
             tc.tile_pool(name="pbc", bufs=2, space="PSUM") as pbc:

            # ---- startup: first tile's x blocks + w0 first, fanned across
            # four idle engine queues so the first matmul can start ASAP ----
            _eng3 = [nc.sync, nc.gpsimd, nc.scalar]

            x0 = []
            for k in range(KO):
                t = ap_.tile([128, SEG], dt.float32r, tag="obsT", bufs=8,
                             name=f"x_0_{k}")
                _eng3[k % 3].dma_start(t[:], obsT[k * 128:(k + 1) * 128, 0:SEG])
                x0.append(t)
            w0t = []
            for k in range(KO):
                t = wp.tile([128, HID], dt.float32r, tag=f"w0_{k}")
                _eng3[(k + 1) % 3].dma_start(t[:], w0[k * 128:(k + 1) * 128, :])
                w0t.append(t)

            # small constants on sync
            b0t = wp.tile([128, MH], dt.float32, tag="b0t")
            nc.sync.dma_start(b0t[:], b0d[:])
            b1t = wp.tile([128, MH], dt.float32, tag="b1t")
            nc.sync.dma_start(b1t[:], b1d[:])
            b2t = wp.tile([128, MH], dt.float32, tag="b2t")
            nc.sync.dma_start(b2t[:], b2d[:])
            lnwt = wp.tile([128, MH], dt.float32, tag="lnwt")
            nc.sync.dma_start(lnwt[:], lnwd[:])
            lnbt = wp.tile([128, MH], dt.float32, tag="lnbt")
            nc.sync.dma_start(lnbt[:], lnbd[:])
            bht = wp.tile([128, NHEAD], dt.float32, tag="bht")
            nc.sync.dma_start(bht[:], bhd[:])
            onesk = wp.tile([128, 1], dt.float32r, tag="onesk")
            nc.sync.dma_start(onesk[:], onesd[:])
            onesr = wp.tile([1, 128], dt.float32r, tag="onesr")
            nc.sync.dma_start(onesr[:], onesrd[:])
            epst = wp.tile([1, 1], dt.float32, tag="epst")
            nc.sync.dma_start(epst[:], epsd[:])

            w1t = [wp.tile([128, HID], dt.float32r, tag=f"w1_{k}",
                           name=f"w1_{k}") for k in range(KH)]
            w2t = [wp.tile([128, HID], dt.float32r, tag=f"w2_{k}",
                           name=f"w2_{k}") for k in range(KH)]

            def _load_deep_weights():
                # w1 needed ~t+20us, w2 ~t+27us; spread queues
                for k in range(KH):
                    nc.sync.dma_start(w1t[k][:], w1[k * 128:(k + 1) * 128, :])
                for k in range(KH):
                    nc.gpsimd.dma_start(w2t[k][:], w2[k * 128:(k + 1) * 128, :])

            def _load_head(t):
                """Stream this tile's head weight k-blocks (tiny, JIT)."""
                slots = [t] if t < NT - 1 else [t, NT]
                whk = []
                for s_ in slots:
                    for k in range(KH):
                        w = ap_.tile([128, A2], dt.float32r, tag="whk", bufs=24,
                                     name=f"whk_{t}_{s_}_{k}")
                        nc.sync.dma_start(
                            w[:], wh[k * 128:(k + 1) * 128,
                                     s_ * A2:(s_ + 1) * A2])
                        whk.append(w)
                return whk

            def emit_l0(t):
                """DMA x, L0 matmuls, bias evict, squares; returns state."""
                c0 = t * SEG
                if t == 0:
                    xk = x0
                else:
                    xk = []
                    for k in range(KO):
                        xt = ap_.tile([128, SEG], dt.float32r, tag="obsT",
                                      bufs=8, name=f"x_{t}_{k}")
                        nc.gpsimd.dma_start(
                            xt[:], obsT[k * 128:(k + 1) * 128, c0:c0 + SEG])
                        xk.append(xt)
                if t == 0:
                    _load_deep_weights()
                whk = _load_head(t)
                h0 = []
                sqa = []
                for half in range(2):
                    acc = ap_.tile([128, SEG], dt.float32r, tag="sqacc", bufs=4,
                                   name=f"sqacc_{t}_{half}")
                    sqa.append(acc)
                for m in range(MH):
                    p = pm.tile([128, SEG], dt.float32, tag="pm", bufs=6,
                                name=f"p0_{t}_{m}")
                    for k in range(KO):
                        nc.tensor.matmul(p[:], w0t[k][:, m * 128:(m + 1) * 128],
                                         xk[k][:], start=(k == 0), stop=(k == KO - 1))
                    h = ap_.tile([128, SEG], dt.float32, tag="h0", bufs=10,
                                 name=f"h0_{t}_{m}")
                    nc.vector.tensor_scalar_add(h[:], p[:], b0t[:, m:m + 1])
                    h0.append(h)
                    acc = sqa[m // 4]
                    if m % 4 == 0:
                        nc.vector.tensor_tensor(acc[:], h[:], h[:], AluOpType.mult)
                    else:
                        sq = ap_.tile([128, SEG], dt.float32, tag="sqt", bufs=2,
                                      name=f"sq_{t}_{m}")
                        nc.vector.tensor_tensor(sq[:], h[:], h[:], AluOpType.mult)
                        nc.vector.tensor_tensor(acc[:], acc[:], sq[:], AluOpType.add)
                return dict(t=t, h0=h0, sqa=sqa, whk=whk)

            def emit_pss(cur):
                t, sqa = cur["t"], cur["sqa"]
                pss = pm.tile([1, SEG], dt.float32, tag="pm", bufs=6,
                              name=f"pss_{t}")
                nc.tensor.matmul(pss[:], onesk[:], sqa[0][:], start=True, stop=False)
                nc.tensor.matmul(pss[:], onesk[:], sqa[1][:], start=False, stop=True)
                cur["pss"] = pss

            def emit_rstd(cur):
                t, pss = cur["t"], cur["pss"]
                # sd = sqrt(pss/HID + eps)
                sd = ap_.tile([1, SEG], dt.float32, tag="rows", bufs=4,
                              name=f"sd_{t}")
                nc.scalar.activation(sd[:], pss[:], AF.Sqrt,
                                     bias=epst[0:1, 0:1], scale=1.0 / HID)
                rstd = ap_.tile([1, SEG], dt.float32, tag="rows", bufs=4,
                                name=f"rstd_{t}")
                nc.vector.reciprocal_approx_fast(rstd[:], sd[:])
                rstd_r = ap_.tile([1, SEG], dt.float32r, tag="rowsr", bufs=2,
                                  name=f"rstdr_{t}")
                nc.scalar.copy(rstd_r[:], rstd[:])
                pR = pbc.tile([128, SEG], dt.float32, tag="pbc", name=f"pR_{t}")
                nc.tensor.matmul(pR[:], onesr[:], rstd_r[:], start=True, stop=True)
                cur["pR"] = pR

            def emit_ln(cur):
                t, h0, pR = cur["t"], cur["h0"], cur["pR"]
                h0n = []
                for m in range(MH):
                    c = ap_.tile([128, SEG], dt.float32, tag="cd", bufs=6,
                                 name=f"c_{t}_{m}")
                    nc.vector.tensor_tensor(c[:], h0[m][:], pR[:], AluOpType.mult)
                    hn = ap_.tile([128, SEG], dt.float32r, tag="hx", bufs=16,
                                  name=f"hn_{t}_{m}")
                    nc.scalar.activation(hn[:], c[:], AF.Prelu,
                                         bias=lnbt[:, m:m + 1],
                                         scale=lnwt[:, m:m + 1], alpha=SLOPE)
                    h0n.append(hn)
                return h0n

            def emit_l123(cur, h0n, nxt):
                """L1/L2/head for tile `cur`; interleaves the pss/rstd
                computation for tile `nxt` behind the first L1 blocks so its
                cross-engine latency hides under PE work."""
                t = cur["t"]
                h1 = []
                for m in range(MH):
                    p = pm.tile([128, SEG], dt.float32, tag="pm", bufs=6,
                                name=f"p1_{t}_{m}")
                    for k in range(KH):
                        nc.tensor.matmul(p[:], w1t[k][:, m * 128:(m + 1) * 128],
                                         h0n[k][:], start=(k == 0), stop=(k == KH - 1))
                    h = ap_.tile([128, SEG], dt.float32r, tag="hx", bufs=16,
                                 name=f"h1_{t}_{m}")
                    nc.scalar.activation(h[:], p[:], AF.Prelu,
                                         bias=b1t[:, m:m + 1], scale=1.0, alpha=SLOPE)
                    h1.append(h)
                    if nxt is not None and m == 4:
                        emit_pss(nxt)
                    if nxt is not None and m == 6:
                        emit_rstd(nxt)
                h2 = []
                for m in range(MH):
                    p = pm.tile([128, SEG], dt.float32, tag="pm", bufs=6,
                                name=f"p2_{t}_{m}")
                    for k in range(KH):
                        nc.tensor.matmul(p[:], w2t[k][:, m * 128:(m + 1) * 128],
                                         h1[k][:], start=(k == 0), stop=(k == KH - 1))
                    h = ap_.tile([128, SEG], dt.float32r, tag="hx", bufs=16,
                                 name=f"h2_{t}_{m}")
                    nc.scalar.activation(h[:], p[:], AF.Prelu,
                                         bias=b2t[:, m:m + 1], scale=1.0, alpha=SLOPE)
                    h2.append(h)
                heads = [t] if t < NT - 1 else [t, NT]
                whk = cur["whk"]
                for hi, s_ in enumerate(heads):
                    p = pm.tile([128, SEG], dt.float32, tag="pm", bufs=6,
                                name=f"ph_{t}_{s_}")
                    for k in range(KH):
                        nc.tensor.matmul(p[:], whk[hi * KH + k][:],
                                         h2[k][:], start=(k == 0), stop=(k == KH - 1))
                    o = ap_.tile([128, SEG], dt.float32, tag="outp", bufs=3,
                                 name=f"o_{t}_{s_}")
                    nc.vector.tensor_scalar_add(o[:], p[:], bht[:, s_:s_ + 1])
                    if s_ == NT:
                        nc.gpsimd.dma_start(out_sec[:, :], o[:])
                    else:
                        c0 = t * SEG
                        nc.gpsimd.dma_start(out_main[:, c0:c0 + SEG], o[:])

            cur = emit_l0(0)
            emit_pss(cur)
            emit_rstd(cur)
            for t in range(NT):
                h0n = emit_ln(cur)
                nxt = emit_l0(t + 1) if t + 1 < NT else None
                emit_l123(cur, h0n, nxt)
                cur = nxt

    nc.compile()
    return nc


def _get_nc():
    if "nc" not in _CACHE:
        _CACHE["nc"] = _build_nc()
    return _CACHE["nc"]


def _pack(stage):
    """Globally sort samples by stage, cut into 64 tiles of 512. Each tile
    spans <=2 stages (asserted). Assign 8 tiles per core with the (at most
    one) boundary-spanning tile placed at slot NT-1 whose secondary head is
    computed into out_sec.

    Returns:
      perm   [NCORES, COLS]  sample index per column
      prim   [NCORES, NT]    primary stage per tile slot
      sec    [NCORES]        secondary stage of slot NT-1 (== prim if pure)
    """
    order = np.argsort(stage, kind="stable")
    tiles = order.reshape(NCORES * NT, SEG)
    tstage = stage[tiles]
    t_first = tstage[:, 0]
    t_last = tstage[:, -1]
    n_distinct = np.array([len(np.unique(r)) for r in tstage])
    if n_distinct.max() > 2:
        raise RuntimeError("a 512-tile spans >2 stages (tiny stage count)")
    mixed = np.where(t_first != t_last)[0]
    pure = np.where(t_first == t_last)[0]
    if len(mixed) > NCORES:
        raise RuntimeError(f"too many mixed tiles: {len(mixed)}")

    perm = np.zeros((NCORES, COLS), np.int64)
    prim = np.zeros((NCORES, NT), np.int64)
    sec = np.zeros(NCORES, np.int64)
    pi = 0
    for c in range(NCORES):
        slots = []
        for s in range(NT - 1):
            slots.append(pure[pi]); pi += 1
        if c < len(mixed):
            slots.append(mixed[c])
        else:
            slots.append(pure[pi]); pi += 1
        for s, tid in enumerate(slots):
            perm[c, s * SEG:(s + 1) * SEG] = tiles[tid]
            prim[c, s] = t_first[tid]
        sec[c] = t_last[slots[-1]]
    return perm, prim, sec


def _prep(inputs):
    obs = np.asarray(inputs["obs"], np.float32)
    stage = np.asarray(inputs["stage"])
    W0 = np.asarray(inputs["W0"], np.float64)
    b0 = np.asarray(inputs["b0"], np.float64)
    ln_w = np.asarray(inputs["ln_w"], np.float32)
    ln_b = np.asarray(inputs["ln_b"], np.float32)
    W1 = np.asarray(inputs["W1"], np.float32)
    b1 = np.asarray(inputs["b1"], np.float32)
    W2 = np.asarray(inputs["W2"], np.float32)
    b2 = np.asarray(inputs["b2"], np.float32)
    W3 = np.asarray(inputs["W3"], np.float64)
    b3 = np.asarray(inputs["b3"], np.float64)
    Wh = np.asarray(inputs["Wh"], np.float64)
    bh = np.asarray(inputs["bh"], np.float64)

    # fold W3 into heads (fp64 for accuracy)
    What = np.einsum("kj,sjo->sko", W3, Wh)          # [S, HID, A2]
    bhat = np.matmul(b3, Wh) + bh                    # [S, A2]

    # fold LN mean subtraction into W0 (rank-1) and b0
    wm = W0.mean(axis=1, keepdims=True)
    W0c = (W0 - wm).astype(np.float32)
    b0c = (b0 - b0.mean()).astype(np.float32)

    perm, prim, sec = _pack(stage)

    shared = {
        "w0": np.ascontiguousarray(W0c),
        "w1": np.ascontiguousarray(W1),
        "w2": np.ascontiguousarray(W2),
        "b0d": np.ascontiguousarray(b0c.reshape(MH, 128).T),
        "b1d": np.ascontiguousarray(b1.reshape(MH, 128).T),
        "b2d": np.ascontiguousarray(b2.reshape(MH, 128).T),
        "lnwd": np.ascontiguousarray(ln_w.reshape(MH, 128).T),
        "lnbd": np.ascontiguousarray(ln_b.reshape(MH, 128).T),
        "onesd": np.ones((128, 1), np.float32),
        "epsd": np.full((1, 1), EPS, np.float32),
        "onesrd": np.ones((1, 128), np.float32),
    }

    What32 = What.astype(np.float32)
    bhat32 = bhat.astype(np.float32)
    in_maps = []
    for c in range(NCORES):
        m = dict(shared)
        m["obsT"] = np.ascontiguousarray(obs[perm[c]].T)
        heads = list(prim[c]) + [sec[c]]
        m["wh"] = np.ascontiguousarray(
            np.concatenate([What32[s] for s in heads], axis=1))
        m["bhd"] = np.ascontiguousarray(bhat32[heads].T)
        in_maps.append(m)
    return in_maps, perm, prim, sec, stage


def _unpack(results, perm, prim, sec, stage):
    out = np.zeros((B, A2), np.float32)
    for c in range(NCORES):
        om = results[c]["out_main"]          # [A2, COLS]
        out[perm[c]] = om.T
        # dual tile: columns whose stage is the secondary head
        lo = (NT - 1) * SEG
        idx = perm[c, lo:lo + SEG]
        mask = stage[idx] != prim[c, NT - 1]
        if mask.any():
            os_ = results[c]["out_sec"]      # [A2, SEG]
            out[idx[mask]] = os_[:, mask].T
    return out


def _run(inputs, trace=False, tmpdir=None):
    nc = _get_nc()
    in_maps, perm, prim, sec, stage = _prep(inputs)
    res = bass_utils.run_bass_kernel_spmd(nc, in_maps, list(range(NCORES)),
                                          trace=trace, tmpdir=tmpdir)
    out = _unpack(res.results, perm, prim, sec, np.asarray(stage))
    mean = np.ascontiguousarray(out[:, :64])
    log_std = np.clip(out[:, 64:], LOG_STD_MIN, LOG_STD_MAX)
    return (mean, log_std), res


def kernel(**inputs):
    (mean, log_std), _ = _run(inputs, trace=False)
    return mean, log_std


def kernel_timed(_tmpdir=None, **inputs):
    (mean, log_std), res = _run(inputs, trace=True, tmpdir=_tmpdir)
    return (mean, log_std), res


# revision 27
# speedup vs baseline: 1.0345x; 1.0345x over previous
"""Trainium2 Bass kernel for nn_GaussianActor (moe_routing).

Strategy (v2):
  - Data parallel over batch across 8 cores; weights replicated except
    per-(core,tile) gathered head weights.
  - Host folds W3 into the per-stage heads (no activation between them):
      What[s] = W3 @ Wh[s],  bhat[s] = b3 @ Wh[s] + bh[s]
  - Host folds the LayerNorm mean subtraction into W0 as a rank-1 update:
      W0c = W0 - rowmean(W0)*1^T, b0c = b0 - mean(b0)
    so h = x @ W0c + b0c is already centered; only the variance needs
    computing on device (squares on vector engine, one ones-vector matmul
    pair on PE, reciprocal_approx_fast + sqrt for rstd).
  - Exact packing: samples globally sorted by stage -> 64 tiles of 512
    columns; <=7 tiles span a stage boundary. Every core gets 8 tiles;
    tile 7 is the (at most one) "dual" tile that computes 2 heads; the
    host gathers per-tile head weights, and selects per-column outputs.
  - Device: feature-major activations (features on partitions, batch on
    free axis), fp32r matmuls, Prelu (parametric relu) activations so the
    scalar engine keeps a single act table (sqrt_and_others) resident.
"""

import ml_dtypes
import numpy as np

import concourse.tile as tile
from concourse import bacc, mybir
from concourse import bass_utils
from concourse.alu_op_type import AluOpType

dt = mybir.dt
AF = mybir.ActivationFunctionType

B = 32768
OBS = 512
HID = 1024
A2 = 128          # 2 * action_dim
NSTAGE = 8
NCORES = 8

SEG = 512         # columns per tile
NT = 8            # tiles per core
COLS = NT * SEG   # 4096 columns per core
NHEAD = NT + 1    # 8 primary head slots + 1 secondary (dual tile)

EPS = 1e-5
SLOPE = 0.01
LOG_STD_MIN, LOG_STD_MAX = -20.0, 2.0

KO = OBS // 128   # 4 k-blocks for layer 0
KH = HID // 128   # 8 k-blocks for hidden layers
MH = HID // 128   # 8 m-blocks of hidden features

_CACHE = {}


def _build_nc():
    nc = bacc.Bacc("TRN2", target_bir_lowering=False, debug=False,
                   num_devices=NCORES)

    obsT = nc.dram_tensor("obsT", [OBS, COLS], dt.bfloat16, kind="ExternalInput").ap()
    w0 = nc.dram_tensor("w0", [OBS, HID], dt.bfloat16, kind="ExternalInput").ap()
    w1 = nc.dram_tensor("w1", [HID, HID], dt.float32r, kind="ExternalInput").ap()
    w2 = nc.dram_tensor("w2", [HID, HID], dt.float32r, kind="ExternalInput").ap()
    wh = nc.dram_tensor("wh", [HID, NHEAD * A2], dt.float32r, kind="ExternalInput").ap()
    b0d = nc.dram_tensor("b0d", [128, MH], dt.float32, kind="ExternalInput").ap()
    b1d = nc.dram_tensor("b1d", [128, MH], dt.float32, kind="ExternalInput").ap()
    b2d = nc.dram_tensor("b2d", [128, MH], dt.float32, kind="ExternalInput").ap()
    lnwd = nc.dram_tensor("lnwd", [128, MH], dt.float32, kind="ExternalInput").ap()
    lnbd = nc.dram_tensor("lnbd", [128, MH], dt.float32, kind="ExternalInput").ap()
    bhd = nc.dram_tensor("bhd", [128, NHEAD], dt.float32, kind="ExternalInput").ap()
    onesd = nc.dram_tensor("onesd", [128, 1], dt.float32r, kind="ExternalInput").ap()
    epsd = nc.dram_tensor("epsd", [1, 1], dt.float32, kind="ExternalInput").ap()
    onesrd = nc.dram_tensor("onesrd", [1, 128], dt.float32r, kind="ExternalInput").ap()

    out_main = nc.dram_tensor("out_main", [A2, COLS], dt.float32,
                              kind="ExternalOutput").ap()
    out_sec = nc.dram_tensor("out_sec", [A2, SEG], dt.float32,
                             kind="ExternalOutput").ap()

    with tile.TileContext(nc) as tc:
        with tc.tile_pool(name="w", bufs=1) as wp, \
             tc.tile_pool(name="acts", bufs=1) as ap_, \
             tc.tile_pool(name="ps", bufs=6, space="PSUM") as pm, \
             tc.tile_pool(name="pbc", bufs=2, space="PSUM") as pbc:

            # ---- startup: first tile's x blocks + w0 first, fanned across
            # four idle engine queues so the first matmul can start ASAP ----
            _eng3 = [nc.sync, nc.gpsimd, nc.scalar]

            x0 = []
            for k in range(KO):
                t = ap_.tile([128, SEG], dt.bfloat16, tag="obsT", bufs=12,
                             name=f"x_0_{k}")
                _eng3[k % 3].dma_start(t[:], obsT[k * 128:(k + 1) * 128, 0:SEG])
                x0.append(t)
            w0t = []
            for k in range(KO):
                t = wp.tile([128, HID], dt.bfloat16, tag=f"w0_{k}")
                _eng3[(k + 1) % 3].dma_start(t[:], w0[k * 128:(k + 1) * 128, :])
                w0t.append(t)

            # small constants on sync
            b0t = wp.tile([128, MH], dt.float32, tag="b0t")
            nc.sync.dma_start(b0t[:], b0d[:])
            b1t = wp.tile([128, MH], dt.float32, tag="b1t")
            nc.sync.dma_start(b1t[:], b1d[:])
            b2t = wp.tile([128, MH], dt.float32, tag="b2t")
            nc.sync.dma_start(b2t[:], b2d[:])
            lnwt = wp.tile([128, MH], dt.float32, tag="lnwt")
            nc.sync.dma_start(lnwt[:], lnwd[:])
            lnbt = wp.tile([128, MH], dt.float32, tag="lnbt")
            nc.sync.dma_start(lnbt[:], lnbd[:])
            bht = wp.tile([128, NHEAD], dt.float32, tag="bht")
            nc.sync.dma_start(bht[:], bhd[:])
            onesk = wp.tile([128, 1], dt.float32r, tag="onesk")
            nc.sync.dma_start(onesk[:], onesd[:])
            onesr = wp.tile([1, 128], dt.float32r, tag="onesr")
            nc.sync.dma_start(onesr[:], onesrd[:])
            epst = wp.tile([1, 1], dt.float32, tag="epst")
            nc.sync.dma_start(epst[:], epsd[:])

            w1t = [wp.tile([128, HID], dt.float32r, tag=f"w1_{k}",
                           name=f"w1_{k}") for k in range(KH)]
            w2t = [wp.tile([128, HID], dt.float32r, tag=f"w2_{k}",
                           name=f"w2_{k}") for k in range(KH)]

            def _load_deep_weights():
                # w1 needed ~t+20us, w2 ~t+27us; spread queues
                for k in range(KH):
                    nc.sync.dma_start(w1t[k][:], w1[k * 128:(k + 1) * 128, :])
                for k in range(KH):
                    nc.gpsimd.dma_start(w2t[k][:], w2[k * 128:(k + 1) * 128, :])

            def _load_head(t):
                """Stream this tile's head weight k-blocks (tiny, JIT)."""
                slots = [t] if t < NT - 1 else [t, NT]
                whk = []
                for s_ in slots:
                    for k in range(KH):
                        w = ap_.tile([128, A2], dt.float32r, tag="whk", bufs=24,
                                     name=f"whk_{t}_{s_}_{k}")
                        nc.sync.dma_start(
                            w[:], wh[k * 128:(k + 1) * 128,
                                     s_ * A2:(s_ + 1) * A2])
                        whk.append(w)
                return whk

            def emit_l0(t):
                """DMA x, L0 matmuls, bias evict, squares; returns state."""
                c0 = t * SEG
                if t == 0:
                    xk = x0
                else:
                    xk = []
                    for k in range(KO):
                        xt = ap_.tile([128, SEG], dt.bfloat16, tag="obsT",
                                      bufs=12, name=f"x_{t}_{k}")
                        nc.gpsimd.dma_start(
                            xt[:], obsT[k * 128:(k + 1) * 128, c0:c0 + SEG])
                        xk.append(xt)
                if t == 0:
                    _load_deep_weights()
                whk = _load_head(t)
                h0 = []
                sqa = []
                for half in range(2):
                    acc = ap_.tile([128, SEG], dt.float32r, tag="sqacc", bufs=4,
                                   name=f"sqacc_{t}_{half}")
                    sqa.append(acc)
                def evict(m, p):
                    h = ap_.tile([128, SEG], dt.float32, tag="h0", bufs=10,
                                 name=f"h0_{t}_{m}")
                    nc.vector.tensor_scalar_add(h[:], p[:], b0t[:, m:m + 1])
                    h0.append(h)
                    acc = sqa[m // 4]
                    if m % 4 == 0:
                        nc.vector.tensor_tensor(acc[:], h[:], h[:], AluOpType.mult)
                    else:
                        sq = ap_.tile([128, SEG], dt.float32, tag="sqt", bufs=2,
                                      name=f"sq_{t}_{m}")
                        nc.vector.tensor_tensor(sq[:], h[:], h[:], AluOpType.mult)
                        nc.vector.tensor_tensor(acc[:], acc[:], sq[:], AluOpType.add)

                if t == 0:
                    # k-outer over m-groups of 3: first matmul needs only
                    # x0[0] + w0[0]; later k blocks arrive while computing
                    for g in range(0, MH, 3):
                        ms = list(range(g, min(g + 3, MH)))
                        ps = [pm.tile([128, SEG], dt.float32, tag="pm", bufs=6,
                                      name=f"p0_{t}_{m}") for m in ms]
                        for k in range(KO):
                            for i, m in enumerate(ms):
                                nc.tensor.matmul(
                                    ps[i][:], w0t[k][:, m * 128:(m + 1) * 128],
                                    xk[k][:], start=(k == 0), stop=(k == KO - 1))
                        for i, m in enumerate(ms):
                            evict(m, ps[i])
                else:
                    for m in range(MH):
                        p = pm.tile([128, SEG], dt.float32, tag="pm", bufs=6,
                                    name=f"p0_{t}_{m}")
                        for k in range(KO):
                            nc.tensor.matmul(
                                p[:], w0t[k][:, m * 128:(m + 1) * 128],
                                xk[k][:], start=(k == 0), stop=(k == KO - 1))
                        evict(m, p)
                return dict(t=t, h0=h0, sqa=sqa, whk=whk)

            def emit_pss(cur):
                t, sqa = cur["t"], cur["sqa"]
                pss = pm.tile([1, SEG], dt.float32, tag="pm", bufs=6,
                              name=f"pss_{t}")
                nc.tensor.matmul(pss[:], onesk[:], sqa[0][:], start=True, stop=False)
                nc.tensor.matmul(pss[:], onesk[:], sqa[1][:], start=False, stop=True)
                cur["pss"] = pss

            def emit_rstd(cur):
                t, pss = cur["t"], cur["pss"]
                # sd = sqrt(pss/HID + eps)
                sd = ap_.tile([1, SEG], dt.float32, tag="rows", bufs=4,
                              name=f"sd_{t}")
                nc.scalar.activation(sd[:], pss[:], AF.Sqrt,
                                     bias=epst[0:1, 0:1], scale=1.0 / HID)
                rstd = ap_.tile([1, SEG], dt.float32, tag="rows", bufs=4,
                                name=f"rstd_{t}")
                nc.vector.reciprocal_approx_fast(rstd[:], sd[:])
                rstd_r = ap_.tile([1, SEG], dt.float32r, tag="rowsr", bufs=2,
                                  name=f"rstdr_{t}")
                nc.scalar.copy(rstd_r[:], rstd[:])
                pR = pbc.tile([128, SEG], dt.float32, tag="pbc", name=f"pR_{t}")
                nc.tensor.matmul(pR[:], onesr[:], rstd_r[:], start=True, stop=True)
                cur["pR"] = pR

            def emit_ln(cur):
                t, h0, pR = cur["t"], cur["h0"], cur["pR"]
                h0n = []
                for m in range(MH):
                    c = ap_.tile([128, SEG], dt.float32, tag="cd", bufs=6,
                                 name=f"c_{t}_{m}")
                    nc.vector.tensor_tensor(c[:], h0[m][:], pR[:], AluOpType.mult)
                    hn = ap_.tile([128, SEG], dt.float32r, tag="hx", bufs=16,
                                  name=f"hn_{t}_{m}")
                    nc.scalar.activation(hn[:], c[:], AF.Prelu,
                                         bias=lnbt[:, m:m + 1],
                                         scale=lnwt[:, m:m + 1], alpha=SLOPE)
                    h0n.append(hn)
                return h0n

            def emit_l123(cur, h0n, nxt):
                """L1/L2/head for tile `cur`; interleaves the pss/rstd
                computation for tile `nxt` behind the first L1 blocks so its
                cross-engine latency hides under PE work."""
                t = cur["t"]
                h1 = []
                for m in range(MH):
                    p = pm.tile([128, SEG], dt.float32, tag="pm", bufs=6,
                                name=f"p1_{t}_{m}")
                    for k in range(KH):
                        nc.tensor.matmul(p[:], w1t[k][:, m * 128:(m + 1) * 128],
                                         h0n[k][:], start=(k == 0), stop=(k == KH - 1))
                    h = ap_.tile([128, SEG], dt.float32r, tag="hx", bufs=16,
                                 name=f"h1_{t}_{m}")
                    nc.scalar.activation(h[:], p[:], AF.Prelu,
                                         bias=b1t[:, m:m + 1], scale=1.0, alpha=SLOPE)
                    h1.append(h)
                    if nxt is not None and m == 4:
                        emit_pss(nxt)
                    if nxt is not None and m == 6:
                        emit_rstd(nxt)
                h2 = []
                for m in range(MH):
                    p = pm.tile([128, SEG], dt.float32, tag="pm", bufs=6,
                                name=f"p2_{t}_{m}")
                    for k in range(KH):
                        nc.tensor.matmul(p[:], w2t[k][:, m * 128:(m + 1) * 128],
                                         h1[k][:], start=(k == 0), stop=(k == KH - 1))
                    h = ap_.tile([128, SEG], dt.float32r, tag="hx", bufs=16,
                                 name=f"h2_{t}_{m}")
                    nc.scalar.activation(h[:], p[:], AF.Prelu,
                                         bias=b2t[:, m:m + 1], scale=1.0, alpha=SLOPE)
                    h2.append(h)
                heads = [t] if t < NT - 1 else [t, NT]
                whk = cur["whk"]
                for hi, s_ in enumerate(heads):
                    p = pm.tile([128, SEG], dt.float32, tag="pm", bufs=6,
                                name=f"ph_{t}_{s_}")
                    for k in range(KH):
                        nc.tensor.matmul(p[:], whk[hi * KH + k][:],
                                         h2[k][:], start=(k == 0), stop=(k == KH - 1))
                    o = ap_.tile([128, SEG], dt.float32, tag="outp", bufs=3,
                                 name=f"o_{t}_{s_}")
                    nc.vector.tensor_scalar_add(o[:], p[:], bht[:, s_:s_ + 1])
                    if s_ == NT:
                        nc.gpsimd.dma_start(out_sec[:, :], o[:])
                    else:
                        c0 = t * SEG
                        nc.gpsimd.dma_start(out_main[:, c0:c0 + SEG], o[:])

            cur = emit_l0(0)
            emit_pss(cur)
            emit_rstd(cur)
            for t in range(NT):
                h0n = emit_ln(cur)
                nxt = emit_l0(t + 1) if t + 1 < NT else None
                emit_l123(cur, h0n, nxt)
                cur = nxt

    nc.compile()
    return nc


def _get_nc():
    if "nc" not in _CACHE:
        _CACHE["nc"] = _build_nc()
    return _CACHE["nc"]


def _pack(stage):
    """Globally sort samples by stage, cut into 64 tiles of 512. Each tile
    spans <=2 stages (asserted). Assign 8 tiles per core with the (at most
    one) boundary-spanning tile placed at slot NT-1 whose secondary head is
    computed into out_sec.

    Returns:
      perm   [NCORES, COLS]  sample index per column
      prim   [NCORES, NT]    primary stage per tile slot
      sec    [NCORES]        secondary stage of slot NT-1 (== prim if pure)
    """
    order = np.argsort(stage, kind="stable")
    tiles = order.reshape(NCORES * NT, SEG)
    tstage = stage[tiles]
    t_first = tstage[:, 0]
    t_last = tstage[:, -1]
    n_distinct = np.array([len(np.unique(r)) for r in tstage])
    if n_distinct.max() > 2:
        raise RuntimeError("a 512-tile spans >2 stages (tiny stage count)")
    mixed = np.where(t_first != t_last)[0]
    pure = np.where(t_first == t_last)[0]
    if len(mixed) > NCORES:
        raise RuntimeError(f"too many mixed tiles: {len(mixed)}")

    perm = np.zeros((NCORES, COLS), np.int64)
    prim = np.zeros((NCORES, NT), np.int64)
    sec = np.zeros(NCORES, np.int64)
    pi = 0
    for c in range(NCORES):
        slots = []
        for s in range(NT - 1):
            slots.append(pure[pi]); pi += 1
        if c < len(mixed):
            slots.append(mixed[c])
        else:
            slots.append(pure[pi]); pi += 1
        for s, tid in enumerate(slots):
            perm[c, s * SEG:(s + 1) * SEG] = tiles[tid]
            prim[c, s] = t_first[tid]
        sec[c] = t_last[slots[-1]]
    return perm, prim, sec


def _prep(inputs):
    obs = np.asarray(inputs["obs"], np.float32)
    stage = np.asarray(inputs["stage"])
    W0 = np.asarray(inputs["W0"], np.float64)
    b0 = np.asarray(inputs["b0"], np.float64)
    ln_w = np.asarray(inputs["ln_w"], np.float32)
    ln_b = np.asarray(inputs["ln_b"], np.float32)
    W1 = np.asarray(inputs["W1"], np.float32)
    b1 = np.asarray(inputs["b1"], np.float32)
    W2 = np.asarray(inputs["W2"], np.float32)
    b2 = np.asarray(inputs["b2"], np.float32)
    W3 = np.asarray(inputs["W3"], np.float64)
    b3 = np.asarray(inputs["b3"], np.float64)
    Wh = np.asarray(inputs["Wh"], np.float64)
    bh = np.asarray(inputs["bh"], np.float64)

    # fold W3 into heads (fp64 for accuracy)
    What = np.einsum("kj,sjo->sko", W3, Wh)          # [S, HID, A2]
    bhat = np.matmul(b3, Wh) + bh                    # [S, A2]

    # fold LN mean subtraction into W0 (rank-1) and b0
    wm = W0.mean(axis=1, keepdims=True)
    W0c = (W0 - wm).astype(np.float32)
    b0c = (b0 - b0.mean()).astype(np.float32)

    perm, prim, sec = _pack(stage)

    bf16 = ml_dtypes.bfloat16
    shared = {
        "w0": np.ascontiguousarray(W0c.astype(bf16)),
        "w1": np.ascontiguousarray(W1),
        "w2": np.ascontiguousarray(W2),
        "b0d": np.ascontiguousarray(b0c.reshape(MH, 128).T),
        "b1d": np.ascontiguousarray(b1.reshape(MH, 128).T),
        "b2d": np.ascontiguousarray(b2.reshape(MH, 128).T),
        "lnwd": np.ascontiguousarray(ln_w.reshape(MH, 128).T),
        "lnbd": np.ascontiguousarray(ln_b.reshape(MH, 128).T),
        "onesd": np.ones((128, 1), np.float32),
        "epsd": np.full((1, 1), EPS, np.float32),
        "onesrd": np.ones((1, 128), np.float32),
    }

    What16 = What.astype(np.float32)
    bhat32 = bhat.astype(np.float32)
    in_maps = []
    for c in range(NCORES):
        m = dict(shared)
        m["obsT"] = np.ascontiguousarray(obs[perm[c]].T.astype(bf16))
        heads = list(prim[c]) + [sec[c]]
        m["wh"] = np.ascontiguousarray(
            np.concatenate([What16[s] for s in heads], axis=1))
        m["bhd"] = np.ascontiguousarray(bhat32[heads].T)
        in_maps.append(m)
    return in_maps, perm, prim, sec, stage


def _unpack(results, perm, prim, sec, stage):
    out = np.zeros((B, A2), np.float32)
    for c in range(NCORES):
        om = results[c]["out_main"]          # [A2, COLS]
        out[perm[c]] = om.T
        # dual tile: columns whose stage is the secondary head
        lo = (NT - 1) * SEG
        idx = perm[c, lo:lo + SEG]
        mask = stage[idx] != prim[c, NT - 1]
        if mask.any():
            os_ = results[c]["out_sec"]      # [A2, SEG]
            out[idx[mask]] = os_[:, mask].T
    return out


def _run(inputs, trace=False, tmpdir=None):
    nc = _get_nc()
    in_maps, perm, prim, sec, stage = _prep(inputs)
    res = bass_utils.run_bass_kernel_spmd(nc, in_maps, list(range(NCORES)),
                                          trace=trace, tmpdir=tmpdir)
    out = _unpack(res.results, perm, prim, sec, np.asarray(stage))
    mean = np.ascontiguousarray(out[:, :64])
    log_std = np.clip(out[:, 64:], LOG_STD_MIN, LOG_STD_MAX)
    return (mean, log_std), res


def kernel(**inputs):
    (mean, log_std), _ = _run(inputs, trace=False)
    return mean, log_std


def kernel_timed(_tmpdir=None, **inputs):
    (mean, log_std), res = _run(inputs, trace=True, tmpdir=_tmpdir)
    return (mean, log_std), res
